# revision 1
# baseline (speedup 1.0000x reference)
"""Trainium2 Bass kernel for nn_AEFS (embedding gather -> controller MLP ->
softmax -> top-20 field masking -> weighted embeddings -> 3-layer MLP).

Self-contained: kernel(**inputs) -> np.ndarray [B] float32.
Data-parallel over batch across 8 NeuronCores; weights/table replicated.
"""
import sys
import numpy as np

sys.path.insert(0, '/opt/trn_rl_repo')

import ml_dtypes
import concourse.bass as bass
import concourse.mybir as mybir
import concourse.tile as tile
from concourse import bacc
from concourse.bass import IndirectOffsetOnAxis
from concourse.bass_utils import run_bass_kernel_spmd

B, F, V, D, K = 16384, 39, 30000, 16, 20
EPS = 1e-5
NCORES = 8
P = 128
FC = F + 1          # fields padded with one dummy field -> 40
KW = FC * D         # padded feature width = 640
KCH = KW // P       # 5 k-chunks of 128
M0, M0CH = 1024, 8  # mlp0 out features, chunks
M1, M1CH = 512, 4   # mlp1 out features, chunks
F32 = mybir.dt.float32
F32R = mybir.dt.float32r
BF16 = mybir.dt.bfloat16
I32 = mybir.dt.int32
AF = mybir.ActivationFunctionType
ALU = mybir.AluOpType
AX = mybir.AxisListType


def build_program(n_groups=16, gt=1):
    """Build the per-core Bass program. B_local = n_groups*gt*128."""
    ntiles = n_groups * gt
    b_loc = ntiles * P
    gN = gt * P                      # batch columns per group (free dim)

    nc = bacc.Bacc("TRN2", target_bir_lowering=False, debug=False,
                   num_devices=NCORES)

    idx_t = nc.dram_tensor("idx", [b_loc, FC], I32, kind="ExternalInput")
    tbl_t = nc.dram_tensor("table", [F * V, D], F32, kind="ExternalInput")
    wc_t = nc.dram_tensor("wc", [KW, F], F32, kind="ExternalInput")
    w0_t = nc.dram_tensor("w0", [KW, M0], BF16, kind="ExternalInput")
    w1_t = nc.dram_tensor("w1", [M0, M1], BF16, kind="ExternalInput")
    wo_t = nc.dram_tensor("wo", [M1, 1], BF16, kind="ExternalInput")
    sel_t = nc.dram_tensor("sel", [FC, KW], BF16, kind="ExternalInput")
    sc0_t = nc.dram_tensor("sc0", [M0], F32, kind="ExternalInput")
    bs0_t = nc.dram_tensor("bs0", [M0], F32, kind="ExternalInput")
    sc1_t = nc.dram_tensor("sc1", [M1], F32, kind="ExternalInput")
    bs1_t = nc.dram_tensor("bs1", [M1], F32, kind="ExternalInput")
    bo_t = nc.dram_tensor("bo", [1, 1], F32, kind="ExternalInput")
    id32_t = nc.dram_tensor("id32", [P, P], F32, kind="ExternalInput")
    id16_t = nc.dram_tensor("id16", [P, P], BF16, kind="ExternalInput")
    out_t = nc.dram_tensor("out", [b_loc], F32, kind="ExternalOutput")

    with tile.TileContext(nc) as tc:
        import contextlib
        with contextlib.ExitStack() as ctx:
            wp = ctx.enter_context(tc.tile_pool(name="weights", bufs=1))
            emb_p = ctx.enter_context(tc.tile_pool(name="emb", bufs=16))
            xt_p = ctx.enter_context(tc.tile_pool(name="xt", bufs=4))
            z_p = ctx.enter_context(tc.tile_pool(name="z", bufs=4))
            h0_p = ctx.enter_context(tc.tile_pool(name="h0", bufs=4))
            h1_p = ctx.enter_context(tc.tile_pool(name="h1", bufs=4))
            sm_p = ctx.enter_context(tc.tile_pool(name="smalls", bufs=4))
            m8_p = ctx.enter_context(tc.tile_pool(name="m8", bufs=4))
            gt_p = ctx.enter_context(tc.tile_pool(name="gt", bufs=4))
            ps_a = ctx.enter_context(
                tc.tile_pool(name="ps_a", bufs=2, space="PSUM"))
            ps_b = ctx.enter_context(
                tc.tile_pool(name="ps_b", bufs=2, space="PSUM"))
            ps_ht = ctx.enter_context(
                tc.tile_pool(name="ps_ht", bufs=2, space="PSUM"))
            ps_mm = ctx.enter_context(
                tc.tile_pool(name="ps_mm", bufs=2, space="PSUM"))

            # ---- load weights / constants ----
            idx_sb = wp.tile([P, ntiles, FC], I32)
            nc.sync.dma_start(idx_sb[:],
                              idx_t.ap().rearrange("(t p) f -> p t f", p=P))
            wc_sb = wp.tile([P, KCH, F], F32)
            nc.sync.dma_start(wc_sb[:],
                              wc_t.ap().rearrange("(c p) m -> p c m", p=P))
            w0_sb = wp.tile([P, KCH, M0], BF16)
            nc.sync.dma_start(w0_sb[:],
                              w0_t.ap().rearrange("(c p) m -> p c m", p=P))
            w1_sb = wp.tile([P, M0CH, M1], BF16)
            nc.sync.dma_start(w1_sb[:],
                              w1_t.ap().rearrange("(c p) m -> p c m", p=P))
            wo_sb = wp.tile([P, M1CH, 1], BF16)
            nc.sync.dma_start(wo_sb[:],
                              wo_t.ap().rearrange("(c p) m -> p c m", p=P))
            sel_sb = wp.tile([FC, KCH, P], BF16)
            nc.sync.dma_start(sel_sb[:],
                              sel_t.ap().rearrange("f (c p) -> f c p", p=P))
            sc0_sb = wp.tile([P, M0CH], F32)
            nc.sync.dma_start(sc0_sb[:],
                              sc0_t.ap().rearrange("(c p) -> p c", p=P))
            bs0_sb = wp.tile([P, M0CH], F32)
            nc.sync.dma_start(bs0_sb[:],
                              bs0_t.ap().rearrange("(c p) -> p c", p=P))
            sc1_sb = wp.tile([P, M1CH], F32)
            nc.sync.dma_start(sc1_sb[:],
                              sc1_t.ap().rearrange("(c p) -> p c", p=P))
            bs1_sb = wp.tile([P, M1CH], F32)
            nc.sync.dma_start(bs1_sb[:],
                              bs1_t.ap().rearrange("(c p) -> p c", p=P))
            bo_sb = wp.tile([1, 1], F32)
            nc.sync.dma_start(bo_sb[:], bo_t.ap())

            id32 = wp.tile([P, P], F32)
            nc.sync.dma_start(id32[:], id32_t.ap())
            id16 = wp.tile([P, P], BF16)
            nc.sync.dma_start(id16[:], id16_t.ap())

            out_sb = wp.tile([1, b_loc], F32)

            for g in range(n_groups):
                # ---- gather: emb_g[p, t, f, d] = table[idx[g*gt+t, p, f], d]
                emb_g = emb_p.tile([P, gt, FC, D], F32)
                for t in range(gt):
                    for f in range(F):
                        # HW indirect DMA: one row-offset per partition
                        nc.gpsimd.indirect_dma_start(
                            out=emb_g[:, t, f, :],
                            out_offset=None,
                            in_=tbl_t.ap(),
                            in_offset=IndirectOffsetOnAxis(
                                ap=idx_sb[:, g * gt + t, f:f + 1], axis=0),
                        )
                # dummy field 39 -> ones row (bias path for controller)
                nc.vector.memset(emb_g[:, :, F, :], 1.0)

                xt_g = xt_p.tile([P, KCH, gN], F32)
                hr_g = sm_p.tile([P, gt, F], F32)
                es_g = sm_p.tile([P, gt, F], F32)
                ra_g = sm_p.tile([P, gt, F], F32)
                rb_g = sm_p.tile([P, gt, F], F32)
                zs_g = sm_p.tile([P, gt], F32)
                rz_g = sm_p.tile([P, gt], F32)
                gsc_g = sm_p.tile([P, gt, FC], F32)
                g16_g = sm_p.tile([P, gt, FC], BF16)
                gT_g = gt_p.tile([FC, gN], BF16)
                hrT_g = gt_p.tile([F, gN], F32, tag="hrT")

                for t in range(gt):
                    embt = emb_g[:, t, :, :].rearrange("p f d -> p (f d)")
                    # transpose 5 chunks of [128,128] -> X_T (feature-major)
                    psa = ps_a.tile([P, 512], F32, tag="pa")
                    for c in range(4):
                        nc.tensor.transpose(
                            out=psa[:, c * P:(c + 1) * P],
                            in_=embt[:, c * P:(c + 1) * P],
                            identity=id32[:])
                    psb = ps_b.tile([P, P], F32, tag="pb")
                    nc.tensor.transpose(
                        out=psb[:], in_=embt[:, 4 * P:5 * P], identity=id32[:])
                    nc.vector.tensor_copy(
                        xt_g[:, 0:4, t * P:(t + 1) * P], psa[:].rearrange(
                            "p (c q) -> p c q", q=P))
                    nc.vector.tensor_copy(xt_g[:, 4, t * P:(t + 1) * P],
                                          psb[:])

                # controller matmul (f32, feature-major, exact): hT[39, gN]
                pht = ps_ht.tile([F, gN], F32, tag="ht")
                for c in range(KCH):
                    nc.tensor.matmul(
                        out=pht[:],
                        lhsT=wc_sb[:, c, :],
                        rhs=xt_g[:, c, :],
                        start=(c == 0), stop=(c == KCH - 1))
                nc.scalar.activation(hrT_g[:], pht[:], AF.Relu)

                # transpose back to batch-major h_relu [128, gt, 39]
                for t in range(gt):
                    # out = in_.T @ I : in_ [39,128] -> out [128, 39]
                    psb = ps_b.tile([P, P], F32, tag="pb")
                    nc.tensor.transpose(
                        out=psb[:, :F],
                        in_=hrT_g[:, t * P:(t + 1) * P],
                        identity=id32[:F, :F])
                    nc.vector.tensor_copy(hr_g[:, t, :], psb[:, :F])

                # softmax pieces: es = exp(h_relu) (ties at h=0 stay exact)
                nc.scalar.activation(
                    es_g[:].rearrange("p t f -> p (t f)"),
                    hr_g[:].rearrange("p t f -> p (t f)"), AF.Exp)
                nc.vector.reduce_sum(out=zs_g[:], in_=es_g[:], axis=AX.X)
                nc.vector.reciprocal(rz_g[:], zs_g[:])

                # top-20 selection on exact f32 h_relu (8+8+4, jax tie order)
                for t in range(gt):
                    m8a = m8_p.tile([P, 8], F32)
                    nc.vector.max(m8a[:], hr_g[:, t, :])
                    nc.vector.match_replace(
                        out=ra_g[:, t, :], in_to_replace=m8a[:],
                        in_values=hr_g[:, t, :], imm_value=-1.0)
                    m8b = m8_p.tile([P, 8], F32)
                    nc.vector.max(m8b[:], ra_g[:, t, :])
                    nc.vector.match_replace(
                        out=rb_g[:, t, :], in_to_replace=m8b[:],
                        in_values=ra_g[:, t, :], imm_value=-1.0)
                    m8c = m8_p.tile([P, 8], F32)
                    nc.vector.max(m8c[:], rb_g[:, t, :])
                    nc.vector.memset(m8c[:, K - 16:], -1.0)
                    nc.vector.match_replace(
                        out=ra_g[:, t, :], in_to_replace=m8c[:],
                        in_values=rb_g[:, t, :], imm_value=-1.0)

                # gscore = es * (1/Z) where selected (ra<0), else 0; dummy -> 1
                nc.vector.scalar_tensor_tensor(
                    out=rb_g[:], in0=ra_g[:], scalar=0.0, op0=ALU.is_lt,
                    op1=ALU.mult, in1=es_g[:])
                rzb = rz_g[:].unsqueeze(2).to_broadcast([P, gt, F])
                nc.vector.tensor_tensor(
                    out=gsc_g[:, :, :F], in0=rb_g[:], in1=rzb, op=ALU.mult)
                nc.vector.memset(gsc_g[:, :, F:], 1.0)
                nc.vector.tensor_copy(g16_g[:], gsc_g[:])

                # transpose gate scores to [40, gN]
                for t in range(gt):
                    psb = ps_b.tile([P, P], BF16, tag="pb")
                    nc.tensor.transpose(
                        out=psb[:FC, :], in_=g16_g[:, t, :], identity=id16[:])
                    nc.vector.tensor_copy(
                        gT_g[:, t * P:(t + 1) * P], psb[:FC, :])

                # Z_T = X_T * replicate(gate): G_rep via selector matmul
                z_g = z_p.tile([P, KCH, gN], BF16)
                for c in range(KCH):
                    gps = ps_mm.tile([P, gN], F32, tag="mm")
                    nc.tensor.matmul(
                        out=gps[:], lhsT=sel_sb[:, c, :], rhs=gT_g[:],
                        start=True, stop=True)
                    nc.vector.tensor_tensor(
                        out=z_g[:, c, :], in0=xt_g[:, c, :], in1=gps[:],
                        op=ALU.mult)

                # MLP0: h0 = relu(sc0 * (W0 @ Z) + bs0), feature-major
                h0_g = h0_p.tile([P, M0CH, gN], BF16)
                for m in range(M0CH):
                    p0 = ps_mm.tile([P, gN], F32, tag="mm")
                    for c in range(KCH):
                        nc.tensor.matmul(
                            out=p0[:],
                            lhsT=w0_sb[:, c, m * P:(m + 1) * P],
                            rhs=z_g[:, c, :],
                            start=(c == 0), stop=(c == KCH - 1))
                    nc.scalar.activation(
                        h0_g[:, m, :], p0[:], AF.Relu,
                        bias=bs0_sb[:, m:m + 1], scale=sc0_sb[:, m:m + 1])

                # MLP1
                h1_g = h1_p.tile([P, M1CH, gN], BF16)
                for m in range(M1CH):
                    p1 = ps_mm.tile([P, gN], F32, tag="mm")
                    for c in range(M0CH):
                        nc.tensor.matmul(
                            out=p1[:],
                            lhsT=w1_sb[:, c, m * P:(m + 1) * P],
                            rhs=h0_g[:, c, :],
                            start=(c == 0), stop=(c == M0CH - 1))
                    nc.scalar.activation(
                        h1_g[:, m, :], p1[:], AF.Relu,
                        bias=bs1_sb[:, m:m + 1], scale=sc1_sb[:, m:m + 1])

                # output layer: o[1, gN]; sigmoid = 1/(1+exp(-(o+b)))
                po = ps_ht.tile([1, gN], F32, tag="ht")
                for c in range(M1CH):
                    nc.tensor.matmul(
                        out=po[:], lhsT=wo_sb[:, c, :],
                        rhs=h1_g[:, c, :],
                        start=(c == 0), stop=(c == M1CH - 1))
                e_sb = sm_p.tile([1, gN], F32)
                nc.scalar.activation(e_sb[:], po[:], AF.Exp,
                                     bias=bo_sb[:], scale=-1.0)
                d_sb = sm_p.tile([1, gN], F32)
                nc.vector.tensor_scalar_add(d_sb[:], e_sb[:], 1.0)
                nc.vector.reciprocal(
                    out=out_sb[:, g * gN:(g + 1) * gN], in_=d_sb[:])
                nc.sync.dma_start(
                    out_t.ap().unsqueeze(0)[:, g * gN:(g + 1) * gN],
                    out_sb[:, g * gN:(g + 1) * gN])

    nc.compile()
    return nc


def prepare_inputs(x, emb_table, w_c, b_c, g_c, be_c, w0, b0, g0, be0,
                   w1, b1, g1, be1, w_out, b_out, n_groups=16, gt=1):
    """Host-side preprocessing -> per-core input maps."""
    x = np.asarray(x)
    b_loc = n_groups * gt * P
    offs = (np.arange(F, dtype=np.int64) * V)
    idx = (x.astype(np.int64) + offs[None, :]).astype(np.int32)   # [B, F]
    idx = np.concatenate([idx, np.zeros((idx.shape[0], 1), np.int32)], axis=1)

    # f-major permutation of the 624 flat features
    perm = np.arange(F * D).reshape(D, F).T.reshape(-1)           # [624]
    rsq = np.float32(1.0 / np.sqrt(1.0 + EPS))

    # controller: fold bn scale into weights, bias into ones-row (row 624)
    sc_c = (np.asarray(g_c) * rsq).astype(np.float32)             # [39]
    bi_c = (np.asarray(g_c) * np.asarray(b_c) * rsq
            + np.asarray(be_c)).astype(np.float32)
    wc_p = np.asarray(w_c)[:, perm] * sc_c[:, None]               # [39, 624]
    wc_kt = np.zeros((KW, F), np.float32)
    wc_kt[:F * D, :] = wc_p.T
    wc_kt[F * D, :] = bi_c

    w0_kt = np.zeros((KW, M0), np.float32)
    w0_kt[:F * D, :] = np.asarray(w0)[:, perm].T
    w1_kt = np.asarray(w1).T.astype(np.float32)                   # [1024, 512]
    wo_kt = np.asarray(w_out).T.astype(np.float32)                # [512, 1]

    sc0 = (np.asarray(g0) * rsq).astype(np.float32)
    bi0 = (np.asarray(g0) * np.asarray(b0) * rsq + np.asarray(be0)).astype(np.float32)
    sc1 = (np.asarray(g1) * rsq).astype(np.float32)
    bi1 = (np.asarray(g1) * np.asarray(b1) * rsq + np.asarray(be1)).astype(np.float32)

    sel = np.zeros((FC, KW), np.float32)
    q = np.arange(KW)
    sel[q // D, q] = 1.0

    bf = ml_dtypes.bfloat16
    common = {
        "table": np.ascontiguousarray(np.asarray(emb_table, np.float32)),
        "wc": wc_kt,
        "w0": w0_kt.astype(bf),
        "w1": w1_kt.astype(bf),
        "wo": wo_kt.astype(bf),
        "sel": sel.astype(bf),
        "sc0": sc0, "bs0": bi0, "sc1": sc1, "bs1": bi1,
        "bo": np.asarray(-np.asarray(b_out, np.float32)).reshape(1, 1),
        "id32": np.eye(P, dtype=np.float32),
        "id16": np.eye(P, dtype=np.float32).astype(bf),
    }
    in_maps = []
    for cix in range(NCORES):
        m = dict(common)
        m["idx"] = np.ascontiguousarray(idx[cix * b_loc:(cix + 1) * b_loc])
        in_maps.append(m)
    return in_maps


_CACHED_NC = None


def kernel(**inputs):
    global _CACHED_NC
    if _CACHED_NC is None:
        _CACHED_NC = build_program()
    nc = _CACHED_NC
    in_maps = prepare_inputs(**inputs)
    res = run_bass_kernel_spmd(nc, in_maps, core_ids=list(range(NCORES)))
    out = np.concatenate([res.results[i]["out"] for i in range(NCORES)])
    return out.astype(np.float32)


if __name__ == "__main__":
    import reference
    ins = {k: np.asarray(v) for k, v in reference.setup_inputs().items()}
    got = kernel(**ins)
    print("out:", got.shape, got.dtype, got[:4])

